# revision 1
# baseline (speedup 1.0000x reference)
"""Causal multi-head attention (B=2, H=16, S=2048, F=128) on 8 TRN2 NeuronCores.

Sharding: tensor-parallel over the (batch, head) axis — 32 independent
(b, h) attention problems, 4 per core. No collectives needed.

Per-head on-chip algorithm (all layouts chosen so no on-chip transposes
are ever required):
  - host pre-transposes x to xT [F, S] per head, and W to WT [f, e].
  - QT = WqT.T @ xT   (PSUM) + bias -> SBUF   [e=128, s=2048]
  - KT likewise.
  - V' = xT_tile.T @ [WvT | 0] + [bv | 1]     [s=128, e'=129] per s-tile
    (extra ones column makes the AV matmul also produce softmax denominators)
  - For each k-tile (128 keys), compute ST = K_tile . Q  ->  [k=128, q<=1024]
    strips in PSUM, exp on ACT -> PT (bf16) in SBUF, lower-triangle zero-mask
    on the diagonal block, then AV: out_acc[q,129] += PT_tile.T @ V'_tile,
    accumulated in PSUM over k-tiles. Column 128 of the accumulator is
    sum(exp) — normalize with DVE reciprocal + tensor_scalar multiply.
  - Causality: only k-tiles with k <= q are ever computed.
"""

import math

import numpy as np
import ml_dtypes

import concourse.bass as bass
import concourse.tile as tile
import concourse.mybir as mybir
from concourse import bacc, bass_utils

B, H, S, F = 2, 16, 2048, 128
NCORES = 8
HPC = (B * H) // NCORES  # (b,h) pairs per core
SCALE = 1.0 / math.sqrt(F)
HALF = S // 2  # q processed in two 1024-wide halves (PSUM budget)
GSTRIDE = 136  # col stride of packed AV accumulator groups (32B aligned)

QK_FP32R = False  # scores/projection matmuls in fp32r (else bf16)

_cache = {}


def _build():
    f32 = mybir.dt.float32
    bf16 = mybir.dt.bfloat16
    f32r = mybir.dt.float32r
    qk_dt = f32r if QK_FP32R else bf16
    Exp = mybir.ActivationFunctionType.Exp

    nc = bacc.Bacc("TRN2")

    if QK_FP32R:
        xtq = nc.dram_tensor("xt32", [HPC, F, S], f32r, kind="ExternalInput")
    xtb = nc.dram_tensor("xtbh", [HPC, F, S], bf16, kind="ExternalInput")
    wqt = nc.dram_tensor("wqt", [HPC, F, F], qk_dt, kind="ExternalInput")
    wkt = nc.dram_tensor("wkt", [HPC, F, F], qk_dt, kind="ExternalInput")
    wvt = nc.dram_tensor("wvt", [HPC, F, F + 1], bf16, kind="ExternalInput")
    bqt = nc.dram_tensor("bqt", [F, HPC], f32, kind="ExternalInput")
    bkt = nc.dram_tensor("bkt", [F, HPC], f32, kind="ExternalInput")
    bvb = nc.dram_tensor("bvb", [HPC, GSTRIDE + F + 1], bf16,
                         kind="ExternalInput")
    msk = nc.dram_tensor("msk", [F, F], bf16, kind="ExternalInput")
    one = nc.dram_tensor("one", [1, F], bf16, kind="ExternalInput")
    out = nc.dram_tensor("out", [HPC, S, F], f32, kind="ExternalOutput")
    if not QK_FP32R:
        xtq = xtb

    with tile.TileContext(nc) as tc, \
            tc.tile_pool(name="consts", bufs=1) as consts, \
            tc.tile_pool(name="xin", bufs=2) as xin, \
            tc.tile_pool(name="qk", bufs=2) as qkp, \
            tc.tile_pool(name="vp", bufs=2 * (S // F)) as vpp, \
            tc.tile_pool(name="pt", bufs=7) as ptp, \
            tc.tile_pool(name="outs", bufs=6) as outp, \
            tc.tile_pool(name="st", bufs=2, space="PSUM") as stp, \
            tc.tile_pool(name="av", bufs=3, space="PSUM") as avp, \
            tc.tile_pool(name="vq", bufs=1, space="PSUM") as vqp:

        c_bq = consts.tile([F, HPC], f32, tag="bq")
        nc.gpsimd.dma_start(out=c_bq, in_=bqt[:, :])
        c_bk = consts.tile([F, HPC], f32, tag="bk")
        nc.gpsimd.dma_start(out=c_bk, in_=bkt[:, :])
        c_mask = consts.tile([F, F], bf16, tag="msk")
        nc.gpsimd.dma_start(out=c_mask, in_=msk[:, :])
        c_one = consts.tile([1, F], bf16, tag="one")
        nc.gpsimd.dma_start(out=c_one, in_=one[:, :])

        # deferred AV-batch emission, two strips deep: by the time an AV
        # batch is emitted, the exp it reads finished ~2 iterations ago,
        # so the PE never stalls waiting on ACT
        SKEW = 5
        pending = []

        def flush_pending(keep=0):
            while len(pending) > keep:
                pending.pop(0)()

        def make_prelude(hd):
            """Emission closures for head hd's input DMAs, QT/KT and V'
            projections. Popped one-per-ki during head hd-1's k-loop so
            this work hides under the previous head's softmax."""
            st8 = {"vtiles": []}

            def dmas(hd=hd):
                # halves so the first QK chunk can start after half a load
                wq = xin.tile([F, F], qk_dt, tag="wq", name=f"wq_{hd}")
                nc.sync.dma_start(out=wq, in_=wqt[hd])
                wk = xin.tile([F, F], qk_dt, tag="wk", name=f"wk_{hd}")
                nc.sync.dma_start(out=wk, in_=wkt[hd])
                xbh = xin.tile([F, S], bf16, tag="xbh", name=f"xbh_{hd}")
                nc.sync.dma_start(out=xbh[:, 0:HALF], in_=xtb[hd][:, 0:HALF])
                nc.sync.dma_start(out=xbh[:, HALF:S], in_=xtb[hd][:, HALF:S])
                if QK_FP32R:
                    x32 = xin.tile([F, S], f32r, tag="x32",
                                   name=f"x32_{hd}")
                    nc.sync.dma_start(out=x32, in_=xtq[hd])
                wv = xin.tile([F, F + 1], bf16, tag="wv", name=f"wv_{hd}")
                nc.sync.dma_start(out=wv, in_=wvt[hd])
                bvr = bvb[hd]
                c_bvb = xin.tile([128, GSTRIDE + F + 1], bf16, tag="bvb",
                                 name=f"bvb_{hd}")
                nc.gpsimd.dma_start(
                    out=c_bvb,
                    in_=bass.AP(tensor=bvr.tensor, offset=bvr.offset,
                                ap=[[0, 128]] + list(bvr.ap)))
                st8["bvb"] = c_bvb
                st8["xbh"], st8["wq"], st8["wk"], st8["wv"] = xbh, wq, wk, wv
                st8["xqk"] = x32 if QK_FP32R else xbh
                st8["qt"] = qkp.tile([F, S], qk_dt, tag="qt",
                                     name=f"qt_{hd}")
                st8["kt"] = qkp.tile([F, S], qk_dt, tag="kt",
                                     name=f"kt_{hd}")

            def qk_chunk(which, c, hd=hd, pool=None, tag="vq", act=False):
                wt, bt = ((st8["wq"], c_bq) if which == "q"
                          else (st8["wk"], c_bk))
                dst = st8["qt" if which == "q" else "kt"]
                ps = (pool or vqp).tile([128, 512], f32, tag=tag,
                                        name=f"qk_{hd}_{which}{c}")
                nc.tensor.matmul(
                    ps[:, 0:512], wt[:, :],
                    st8["xqk"][:, 512 * c:512 * (c + 1)],
                    start=True, stop=True)
                if act:  # startup only: ACT is idle, spare the DVE chain
                    nc.scalar.activation(
                        out=dst[:, 512 * c:512 * (c + 1)], in_=ps[:, 0:512],
                        func=mybir.ActivationFunctionType.Identity,
                        bias=bt[:, hd:hd + 1])
                else:
                    nc.vector.tensor_scalar_add(
                        dst[:, 512 * c:512 * (c + 1)], ps[:, 0:512],
                        bt[:, hd:hd + 1])

            def vpd_tile(j, hd=hd):
                # two s-tiles of V' share one PSUM bank (cols 0 and GSTRIDE)
                # and one SBUF tile + one evacuation copy. The second
                # prefill's start=True clears the whole bank's has_written,
                # but pair A is fully accumulated by then (data persists).
                ps = vqp.tile([128, 512], f32, tag="vq",
                              name=f"vps_{hd}_{j}")
                for half_j in range(2):
                    si = 2 * j + half_j
                    g = GSTRIDE * half_j
                    nc.tensor.matmul(
                        ps[:, g:g + F + 1],
                        st8["xbh"][:, 128 * si:128 * (si + 1)],
                        st8["wv"][:, :],
                        start=True, stop=True, skip_group_check=True)
                vt = vpp.tile([128, GSTRIDE + F + 1], bf16, tag="vp",
                              name=f"vp_{hd}_{j}")
                # evacuation copy with the [bv|1] bias folded in
                nc.vector.scalar_tensor_tensor(
                    out=vt[:, :], in0=ps[:, 0:GSTRIDE + F + 1], scalar=1.0,
                    in1=st8["bvb"][:, :], op0=mybir.AluOpType.mult,
                    op1=mybir.AluOpType.add)
                st8["vtiles"].append(vt[:, 0:F + 1])
                st8["vtiles"].append(vt[:, GSTRIDE:GSTRIDE + F + 1])

            # ordered so V' pairs arrive just ahead of the AV batches that
            # need them, and QT/KT chunks ahead of the halves that read
            # them; 14 closures <= 24 k-iterations, so nothing spills to
            # the next head's boundary
            closures = [dmas]
            if hd == 0:
                # startup: spread the first chunks over idle PSUM pools and
                # both ACT+DVE so they run concurrently
                closures.append(lambda: qk_chunk("q", 0, pool=stp, tag="st"))
                closures.append(lambda: qk_chunk("k", 0, pool=avp, tag="av"))
                closures.append(lambda: qk_chunk("q", 1, pool=stp, tag="st"))
                closures.append(lambda: qk_chunk("k", 1, pool=avp, tag="av"))
                order = [lambda: None, lambda: None,
                         lambda: vpd_tile(0), lambda: vpd_tile(1),
                         lambda c=2: qk_chunk("q", c),
                         lambda c=3: qk_chunk("q", c),
                         lambda c=2: qk_chunk("k", c),
                         lambda c=3: qk_chunk("k", c),
                         lambda: vpd_tile(2), lambda: vpd_tile(3),
                         lambda: vpd_tile(4), lambda: vpd_tile(5),
                         lambda: vpd_tile(6), lambda: vpd_tile(7)]
            else:
                for c in (0, 1):
                    closures.append(lambda c=c: qk_chunk("q", c))
                    closures.append(lambda c=c: qk_chunk("k", c))
                order = [lambda: vpd_tile(0), lambda: vpd_tile(1),
                         lambda c=2: qk_chunk("q", c),
                         lambda c=2: qk_chunk("k", c),
                         lambda: vpd_tile(2),
                         lambda c=3: qk_chunk("q", c),
                         lambda c=3: qk_chunk("k", c),
                         lambda: vpd_tile(3), lambda: vpd_tile(4),
                         lambda: vpd_tile(5), lambda: vpd_tile(6),
                         lambda: vpd_tile(7)]
            closures.extend(order)
            return st8, closures

        head_state = {}
        head_state[0], prelude = make_prelude(0)
        for _ in range(5):  # dmas + q0/k0/q1/k1, on parallel PSUM slots
            prelude.pop(0)()

        for hd in range(HPC):
            if hd > 0:
                while prelude:  # leftovers from the previous k-loop
                    prelude.pop(0)()
            if hd + 1 < HPC:
                head_state[hd + 1], nxt = make_prelude(hd + 1)
                prelude.extend(nxt)
            qt_t = head_state[hd]["qt"]
            kt_t = head_state[hd]["kt"]
            vtiles = head_state[hd]["vtiles"]

            # --- attention, q in two 1024-wide halves ---
            for half in range(2):
                q0 = half * HALF
                nk = (half + 1) * (HALF // 128)  # k-tiles touching this half
                hstate = {}

                for ki in range(nk):
                    ks = 128 * ki
                    ls = max(0, ks - q0)  # local start col within strip
                    strip = stp.tile([128, 1024], f32, tag="st")
                    bounds = [ls, 512, 1024] if ls < 512 else [ls, 1024]
                    pieces = list(zip(bounds[:-1], bounds[1:]))
                    # first ST piece, then the deferred AV batch (whose
                    # matmul stream hides this piece's drain), then the
                    # second piece (its weight load hides under AV drains)
                    nc.tensor.matmul(
                        strip[:, pieces[0][0]:pieces[0][1]],
                        kt_t[:, ks:ks + 128],
                        qt_t[:, q0 + pieces[0][0]:q0 + pieces[0][1]],
                        start=True, stop=True)
                    if prelude:  # hide next head's QKV/V' here
                        prelude.pop(0)()
                    flush_pending(keep=SKEW - 1)
                    for c0, c1 in pieces[1:]:
                        nc.tensor.matmul(
                            strip[:, c0:c1], kt_t[:, ks:ks + 128],
                            qt_t[:, q0 + c0:q0 + c1],
                            start=True, stop=True)
                    ptile = ptp.tile([128, 1024], bf16, tag="pt")
                    nc.scalar.activation(
                        out=ptile[:, ls:1024], in_=strip[:, ls:1024],
                        func=Exp, scale=SCALE)
                    if ks >= q0:  # zero the below-diagonal of the diag block
                        nc.vector.tensor_mul(
                            ptile[:, ls:ls + 128], ptile[:, ls:ls + 128],
                            c_mask[:, :])

                    def av_batch(hd=hd, half=half, ki=ki, ptile=ptile,
                                 hstate=hstate, vtiles=vtiles):
                        if ki == 0:
                            # start=True clears has_written for the WHOLE
                            # bank (per partition), so only the FIRST
                            # matmul into each bank (qt%3==0 at ki=0) may
                            # carry it; the other packed groups' first
                            # writes find their bits clear and overwrite.
                            hstate["avts"] = [
                                avp.tile([128, 512], f32, tag="av",
                                         name=f"avacc_{hd}_{half}_{i}")
                                for i in range(3)]
                        avts = hstate["avts"]
                        for qt in range(max(0, ki - 8 * half), 8):
                            qg = 8 * half + qt
                            g = GSTRIDE * (qt % 3)
                            acc = avts[qt // 3][:, g:g + F + 1]
                            nc.tensor.matmul(
                                acc, ptile[:, 128 * qt:128 * qt + 128],
                                vtiles[ki][:, :],
                                start=(ki == 0 and qt % 3 == 0),
                                stop=(ki == qg),
                                skip_group_check=True)
                        # normalize + store once a whole accumulator bank
                        # is finished (avoids PE-write/DVE-read bank overlap)
                        for bank in range(3):
                            last_qt = min(3 * bank + 2, 7)
                            if ki != 8 * half + last_qt:
                                continue
                            ng = last_qt - 3 * bank + 1
                            rc = outp.tile([128, 3], f32, tag="rc")
                            # one strided reciprocal over the bank's sum
                            # columns (at F, F+GSTRIDE, ...)
                            nc.vector.reciprocal(
                                rc[:, 0:ng],
                                avts[bank][:, F:F + 1 + GSTRIDE * (ng - 1):
                                           GSTRIDE])
                            for qt in range(3 * bank, last_qt + 1):
                                qg = 8 * half + qt
                                g = GSTRIDE * (qt % 3)
                                acc = avts[bank][:, g:g + F + 1]
                                ot = outp.tile([128, F], f32, tag="ot")
                                nc.vector.tensor_scalar_mul(
                                    ot[:, :], acc[:, 0:F],
                                    rc[:, qt % 3:qt % 3 + 1])
                                nc.sync.dma_start(
                                    out=out[hd, 128 * qg:128 * (qg + 1), :],
                                    in_=ot[:, :])

                    pending.append(av_batch)
        flush_pending()

    nc.compile()
    return nc


def _prep_inputs(x, Wq, Wk, Wv, bq, bk, bv):
    """Shard + pre-transpose on host. Returns in_maps for 8 cores."""
    bf16 = ml_dtypes.bfloat16
    xf = np.ascontiguousarray(
        x.reshape(B * H, S, F).transpose(0, 2, 1)).astype(np.float32)  # [32,F,S]
    xfb = xf.astype(bf16)
    wqT = np.ascontiguousarray(Wq.transpose(0, 2, 1)).astype(np.float32)  # [H,f,e]
    wkT = np.ascontiguousarray(Wk.transpose(0, 2, 1)).astype(np.float32)
    wvT = np.ascontiguousarray(Wv.transpose(0, 2, 1)).astype(np.float32)
    wvTp = np.zeros((H, F, F + 1), np.float32)
    wvTp[:, :, :F] = wvT
    wvTp = wvTp.astype(bf16)
    bvb_h = np.zeros((H, GSTRIDE + F + 1), np.float32)
    bvb_h[:, 0:F] = bv
    bvb_h[:, F] = 1.0
    bvb_h[:, GSTRIDE:GSTRIDE + F] = bv
    bvb_h[:, GSTRIDE + F] = 1.0
    mask = np.triu(np.ones((F, F), np.float32)).astype(bf16)  # keep r <= c
    ones_row = np.ones((1, F), np.float32).astype(bf16)

    wq_dt = np.float32 if QK_FP32R else bf16
    in_maps = []
    for c in range(NCORES):
        pairs = list(range(HPC * c, HPC * (c + 1)))
        heads = [p % H for p in pairs]
        m = {
            "xtbh": np.ascontiguousarray(xfb[pairs]),
            "wqt": np.ascontiguousarray(wqT[heads]).astype(wq_dt),
            "wkt": np.ascontiguousarray(wkT[heads]).astype(wq_dt),
            "wvt": np.ascontiguousarray(wvTp[heads]),
            "bqt": np.ascontiguousarray(bq[heads].T).astype(np.float32),
            "bkt": np.ascontiguousarray(bk[heads].T).astype(np.float32),
            "bvb": np.ascontiguousarray(bvb_h[heads]).astype(bf16),
            "msk": mask,
            "one": ones_row,
        }
        if QK_FP32R:
            m["xt32"] = np.ascontiguousarray(xf[pairs])
        in_maps.append(m)
    return in_maps


def kernel(x, Wq, Wk, Wv, bq, bk, bv, trace=False):
    x, Wq, Wk, Wv = (np.asarray(a, np.float32) for a in (x, Wq, Wk, Wv))
    bq, bk, bv = (np.asarray(a, np.float32) for a in (bq, bk, bv))

    if "nc" not in _cache:
        _cache["nc"] = _build()
    nc = _cache["nc"]

    in_maps = _prep_inputs(x, Wq, Wk, Wv, bq, bk, bv)
    res = bass_utils.run_bass_kernel_spmd(
        nc, in_maps, core_ids=list(range(NCORES)), trace=trace)

    out = np.empty((B * H, S, F), np.float32)
    for c in range(NCORES):
        out[HPC * c:HPC * (c + 1)] = res.results[c]["out"]
    full = out.reshape(B, H, S, F)
    if trace:
        return full, res
    return full



# revision 4
# speedup vs baseline: 1.0058x; 1.0058x over previous
"""Causal multi-head attention (B=2, H=16, S=2048, F=128) on 8 TRN2 NeuronCores.

Sharding: tensor-parallel over the (batch, head) axis — 32 independent
(b, h) attention problems, 4 per core. No collectives needed.

Score algebra (per head): with M = Wq^T Wk, u = Wq^T bk, w = Wk^T bq,
  s[q,k] = (x_q Wq^T + bq)·(x_k Wk^T + bk)
         = x_q·(M x_k + u) + w·x_k + const
and the per-head const drops inside softmax.  So instead of projecting
Q and K separately (two matmul passes), the kernel computes a single
z = M^T-stationary projection (z_k = M x_k + u), and the per-key scalar
beta_k = w·x_k rides along the V projection as an extra moving column.
beta is applied inside the exp via the ACT per-partition bias operand.

Per-head on-chip algorithm (no on-chip transposes):
  - host pre-transposes x to xT [F, S] and supplies mt = M^T [f,f'],
    wvx = [Wv^T | 0 | SCALE*w] [f, F+2].
  - ZT = mt.T @ xT (PSUM) + u -> SBUF [f', s]   (one pass, not two)
  - V' = xT_tile.T @ wvx per s-tile -> [s, F+2]: cols 0..127 V, col 128
    becomes the softmax-denominator ones column (bias add), col 129 is
    SCALE*beta for that key tile.
  - For each k-tile: ST strip = Z_tile . X -> [k, q<=1024] in PSUM,
    exp(SCALE*st + SCALE*beta) on ACT -> PT bf16 in SBUF, triangle mask
    on the diagonal block, then AV: acc[q, 129] += PT_tile.T @ V'_tile
    accumulated over k-tiles in PSUM.  Column 128 = sum(exp).
  - Finished accumulator groups are DMA'd straight from PSUM to DRAM
    UNNORMALIZED; the host divides by the denominator column and adds
    bv (out = acc/den + bv, exact because sum_k P·bv = den·bv).
"""

import math

import numpy as np
import ml_dtypes

import concourse.bass as bass
import concourse.tile as tile
import concourse.mybir as mybir
from concourse import bacc, bass_utils

B, H, S, F = 2, 16, 2048, 128
NCORES = 8
HPC = (B * H) // NCORES  # (b,h) pairs per core
SCALE = 1.0 / math.sqrt(F)
HALF = S // 2  # q processed in two 1024-wide halves (PSUM budget)
NKT = S // F   # 16 k-tiles per head
GSTRIDE = 136  # col stride of packed PSUM groups (32B aligned)
VW = F + 2     # V' projection width: [Wv | denom-ones | beta]

_cache = {}


def _build():
    f32 = mybir.dt.float32
    bf16 = mybir.dt.bfloat16
    Exp = mybir.ActivationFunctionType.Exp
    Ident = mybir.ActivationFunctionType.Identity

    nc = bacc.Bacc("TRN2")

    xtb = nc.dram_tensor("xtbh", [HPC, F, S], bf16, kind="ExternalInput")
    mtd = nc.dram_tensor("mtd", [HPC, F, F], bf16, kind="ExternalInput")
    wvx = nc.dram_tensor("wvx", [HPC, F, VW], bf16, kind="ExternalInput")
    ud = nc.dram_tensor("ud", [F, HPC], f32, kind="ExternalInput")
    vb1 = nc.dram_tensor("vb1", [1, GSTRIDE + VW], bf16, kind="ExternalInput")
    msk = nc.dram_tensor("msk", [F, F], bf16, kind="ExternalInput")
    out = nc.dram_tensor("out", [HPC, NKT, F, F + 1], f32,
                         kind="ExternalOutput")

    with tile.TileContext(nc) as tc, \
            tc.tile_pool(name="consts", bufs=1) as consts, \
            tc.tile_pool(name="xin", bufs=2) as xin, \
            tc.tile_pool(name="zt", bufs=2) as ztp, \
            tc.tile_pool(name="vp", bufs=2 * NKT) as vpp, \
            tc.tile_pool(name="pt", bufs=7) as ptp, \
            tc.tile_pool(name="outs", bufs=4) as outp, \
            tc.tile_pool(name="st", bufs=2, space="PSUM") as stp, \
            tc.tile_pool(name="av", bufs=3, space="PSUM") as avp, \
            tc.tile_pool(name="vq", bufs=1, space="PSUM") as vqp:

        c_u = consts.tile([F, HPC], f32, tag="u")
        nc.gpsimd.dma_start(out=c_u, in_=ud[:, :])
        c_mask = consts.tile([F, F], bf16, tag="msk")
        nc.gpsimd.dma_start(out=c_mask, in_=msk[:, :])
        # V'-evac bias, broadcast to all partitions: 1.0 at the two
        # denominator columns (128 and GSTRIDE+128), 0 elsewhere
        c_vb = consts.tile([128, GSTRIDE + VW], bf16, tag="vb")
        vbr = vb1[0]
        nc.gpsimd.dma_start(
            out=c_vb,
            in_=bass.AP(tensor=vbr.tensor, offset=vbr.offset,
                        ap=[[0, 128]] + list(vbr.ap)))

        # deferred AV-batch emission: by the time an AV batch is
        # emitted, the exp it reads finished ~2 iterations ago, so the
        # PE never stalls waiting on ACT
        SKEW = 5
        pending = []

        def flush_pending(keep=0):
            while len(pending) > keep:
                pending.pop(0)()

        def make_prelude(hd):
            """Emission closures for head hd's input DMAs, Z and V'
            projections. Popped one-per-ki during head hd-1's k-loop so
            this work hides under the previous head's softmax."""
            st8 = {"vav": [], "vbeta": []}

            def dmas(hd=hd):
                mt = xin.tile([F, F], bf16, tag="mt", name=f"mt_{hd}")
                nc.sync.dma_start(out=mt, in_=mtd[hd])
                xbh = xin.tile([F, S], bf16, tag="xbh", name=f"xbh_{hd}")
                nc.sync.dma_start(out=xbh[:, 0:HALF], in_=xtb[hd][:, 0:HALF])
                nc.sync.dma_start(out=xbh[:, HALF:S], in_=xtb[hd][:, HALF:S])
                wv = xin.tile([F, VW], bf16, tag="wv", name=f"wv_{hd}")
                nc.sync.dma_start(out=wv, in_=wvx[hd])
                st8["xbh"], st8["mt"], st8["wv"] = xbh, mt, wv
                st8["zt"] = ztp.tile([F, S], bf16, tag="zt", name=f"zt_{hd}")

            def z_chunk(c, hd=hd, pool=None, tag="vq", act=False):
                ps = (pool or vqp).tile([128, 512], f32, tag=tag,
                                        name=f"z_{hd}_{c}")
                nc.tensor.matmul(
                    ps[:, 0:512], st8["mt"][:, :],
                    st8["xbh"][:, 512 * c:512 * (c + 1)],
                    start=True, stop=True)
                dst = st8["zt"][:, 512 * c:512 * (c + 1)]
                if act:  # startup only: ACT is idle then
                    nc.scalar.activation(out=dst, in_=ps[:, 0:512],
                                         func=Ident, bias=c_u[:, hd:hd + 1])
                else:
                    nc.vector.tensor_scalar_add(dst, ps[:, 0:512],
                                                c_u[:, hd:hd + 1])

            def vpd_tile(j, hd=hd):
                # two s-tiles of V' share one PSUM bank (cols 0 and
                # GSTRIDE) and one SBUF tile + one evacuation copy. The
                # second prefill's start=True clears the whole bank's
                # has_written, but pair A is fully written by then
                # (data persists).
                ps = vqp.tile([128, 512], f32, tag="vq",
                              name=f"vps_{hd}_{j}")
                for half_j in range(2):
                    si = 2 * j + half_j
                    g = GSTRIDE * half_j
                    nc.tensor.matmul(
                        ps[:, g:g + VW],
                        st8["xbh"][:, 128 * si:128 * (si + 1)],
                        st8["wv"][:, :],
                        start=True, stop=True, skip_group_check=True)
                vt = vpp.tile([128, GSTRIDE + VW], bf16, tag="vp",
                              name=f"vp_{hd}_{j}")
                # evacuation copy with the denominator-ones column
                # folded in via the broadcast bias tile
                nc.vector.scalar_tensor_tensor(
                    out=vt[:, :], in0=ps[:, 0:GSTRIDE + VW], scalar=1.0,
                    in1=c_vb[:, :], op0=mybir.AluOpType.mult,
                    op1=mybir.AluOpType.add)
                for half_j in range(2):
                    g = GSTRIDE * half_j
                    st8["vav"].append(vt[:, g:g + F + 1])
                    st8["vbeta"].append(vt[:, g + F + 1:g + F + 2])

            # ordered so V' pairs arrive ahead of the exps that read
            # their beta column, and Z chunks ahead of the strips that
            # read them; 13 closures <= 24 k-iterations
            closures = [dmas]
            if hd == 0:
                # startup: spread the first chunks over idle PSUM pools
                # and both ACT+DVE so they run concurrently
                closures.append(lambda: z_chunk(0, pool=stp, tag="st",
                                                act=True))
                closures.append(lambda: z_chunk(1, pool=avp, tag="av"))
                closures.append(lambda: vpd_tile(0))
                order = [lambda: z_chunk(2), lambda: vpd_tile(1),
                         lambda: z_chunk(3), lambda: vpd_tile(2),
                         lambda: vpd_tile(3), lambda: vpd_tile(4),
                         lambda: vpd_tile(5), lambda: vpd_tile(6),
                         lambda: vpd_tile(7)]
            else:
                closures.append(lambda: z_chunk(0))
                closures.append(lambda: z_chunk(1))
                closures.append(lambda: vpd_tile(0))
                order = [lambda: z_chunk(2), lambda: vpd_tile(1),
                         lambda: z_chunk(3), lambda: vpd_tile(2),
                         lambda: vpd_tile(3), lambda: vpd_tile(4),
                         lambda: vpd_tile(5), lambda: vpd_tile(6),
                         lambda: vpd_tile(7)]
            closures.extend(order)
            return st8, closures

        head_state = {}
        head_state[0], prelude = make_prelude(0)
        for _ in range(4):  # dmas + z0/z1 on parallel PSUM slots + vpd0
            prelude.pop(0)()

        for hd in range(HPC):
            if hd > 0:
                while prelude:  # leftovers from the previous k-loop
                    prelude.pop(0)()
            if hd + 1 < HPC:
                head_state[hd + 1], nxt = make_prelude(hd + 1)
                prelude.extend(nxt)
            zt_t = head_state[hd]["zt"]
            xbh_t = head_state[hd]["xbh"]
            vav = head_state[hd]["vav"]
            vbeta = head_state[hd]["vbeta"]

            # --- attention, q in two 1024-wide halves ---
            for half in range(2):
                q0 = half * HALF
                nk = (half + 1) * (HALF // 128)  # k-tiles for this half
                hstate = {}

                for ki in range(nk):
                    ks = 128 * ki
                    ls = max(0, ks - q0)  # local start col within strip
                    strip = stp.tile([128, 1024], f32, tag="st")
                    bounds = [ls, 512, 1024] if ls < 512 else [ls, 1024]
                    pieces = list(zip(bounds[:-1], bounds[1:]))
                    # first ST piece, then the deferred AV batch (whose
                    # matmul stream hides this piece's drain), then the
                    # second piece (weight load hides under AV drains)
                    nc.tensor.matmul(
                        strip[:, pieces[0][0]:pieces[0][1]],
                        zt_t[:, ks:ks + 128],
                        xbh_t[:, q0 + pieces[0][0]:q0 + pieces[0][1]],
                        start=True, stop=True)
                    if prelude:  # hide next head's Z/V' here
                        prelude.pop(0)()
                    flush_pending(keep=SKEW - 1)
                    for c0, c1 in pieces[1:]:
                        nc.tensor.matmul(
                            strip[:, c0:c1], zt_t[:, ks:ks + 128],
                            xbh_t[:, q0 + c0:q0 + c1],
                            start=True, stop=True)
                    ptile = ptp.tile([128, 1024], bf16, tag="pt")
                    nc.scalar.activation(
                        out=ptile[:, ls:1024], in_=strip[:, ls:1024],
                        func=Exp, scale=SCALE, bias=vbeta[ki])
                    if ks >= q0:  # zero below-diagonal of the diag block
                        nc.vector.tensor_mul(
                            ptile[:, ls:ls + 128], ptile[:, ls:ls + 128],
                            c_mask[:, :])

                    def av_batch(hd=hd, half=half, ki=ki, ptile=ptile,
                                 hstate=hstate, vav=vav):
                        if ki == 0:
                            # start=True clears has_written for the
                            # WHOLE bank (per partition), so only the
                            # FIRST matmul into each bank (qt%3==0 at
                            # ki=0) may carry it; the other packed
                            # groups' first writes find their bits
                            # clear and overwrite.
                            hstate["avts"] = [
                                avp.tile([128, 512], f32, tag="av",
                                         name=f"avacc_{hd}_{half}_{i}")
                                for i in range(3)]
                        avts = hstate["avts"]
                        for qt in range(max(0, ki - 8 * half), 8):
                            qg = 8 * half + qt
                            g = GSTRIDE * (qt % 3)
                            acc = avts[qt // 3][:, g:g + F + 1]
                            nc.tensor.matmul(
                                acc, ptile[:, 128 * qt:128 * qt + 128],
                                vav[ki][:, :],
                                start=(ki == 0 and qt % 3 == 0),
                                stop=(ki == qg),
                                skip_group_check=True)
                        # once a whole accumulator bank is finished,
                        # stage it to SBUF with ONE copy (DMA cannot
                        # read PSUM) and DMA the (unnormalized) groups
                        # out; host divides by the denominator column
                        for bank in range(3):
                            last_qt = min(3 * bank + 2, 7)
                            if ki != 8 * half + last_qt:
                                continue
                            ng = last_qt - 3 * bank + 1
                            w = GSTRIDE * (ng - 1) + F + 1
                            stage = outp.tile([128, 2 * GSTRIDE + F + 1],
                                              f32, tag="ot")
                            nc.vector.tensor_copy(
                                out=stage[:, 0:w],
                                in_=avts[bank][:, 0:w])
                            for qt in range(3 * bank, last_qt + 1):
                                qg = 8 * half + qt
                                g = GSTRIDE * (qt % 3)
                                nc.sync.dma_start(
                                    out=out[hd, qg],
                                    in_=stage[:, g:g + F + 1])

                    pending.append(av_batch)
        flush_pending()

    nc.compile()
    return nc


def _prep_inputs(x, Wq, Wk, Wv, bq, bk, bv):
    """Shard + pre-transpose + fold weights on host. 8 core in_maps."""
    bf16 = ml_dtypes.bfloat16
    xf = np.ascontiguousarray(
        x.reshape(B * H, S, F).transpose(0, 2, 1))          # [32, F, S]
    xfb = xf.astype(bf16)
    # mt = M^T = (Wq^T Wk)^T = Wk^T Wq, per head  [f, f']
    mt = np.einsum("hef,heg->hfg", Wk, Wq).astype(bf16)     # [H, f, g=f']
    u = np.einsum("hef,he->hf", Wq, bk).astype(np.float32)  # [H, f']
    w = np.einsum("hef,he->hf", Wk, bq).astype(np.float32)  # [H, f]
    # wvx = [Wv^T | 0 | SCALE*w]  [f, VW]
    wvxh = np.zeros((H, F, VW), np.float32)
    wvxh[:, :, :F] = Wv.transpose(0, 2, 1)
    wvxh[:, :, F + 1] = SCALE * w
    wvxh = wvxh.astype(bf16)
    vb = np.zeros((1, GSTRIDE + VW), np.float32)
    vb[0, F] = 1.0
    vb[0, GSTRIDE + F] = 1.0
    mask = np.triu(np.ones((F, F), np.float32)).astype(bf16)  # keep r <= c

    in_maps = []
    for c in range(NCORES):
        pairs = list(range(HPC * c, HPC * (c + 1)))
        heads = [p % H for p in pairs]
        m = {
            "xtbh": np.ascontiguousarray(xfb[pairs]),
            "mtd": np.ascontiguousarray(mt[heads]),
            "wvx": np.ascontiguousarray(wvxh[heads]),
            "ud": np.ascontiguousarray(u[heads].T).astype(np.float32),
            "vb1": vb.astype(bf16),
            "msk": mask,
        }
        in_maps.append(m)
    return in_maps


def kernel(x, Wq, Wk, Wv, bq, bk, bv, trace=False):
    x, Wq, Wk, Wv = (np.asarray(a, np.float32) for a in (x, Wq, Wk, Wv))
    bq, bk, bv = (np.asarray(a, np.float32) for a in (bq, bk, bv))

    if "nc" not in _cache:
        _cache["nc"] = _build()
    nc = _cache["nc"]

    in_maps = _prep_inputs(x, Wq, Wk, Wv, bq, bk, bv)
    res = bass_utils.run_bass_kernel_spmd(
        nc, in_maps, core_ids=list(range(NCORES)), trace=trace)

    out = np.empty((B * H, S, F), np.float32)
    for c in range(NCORES):
        pairs = range(HPC * c, HPC * (c + 1))
        r = res.results[c]["out"]  # [HPC, NKT, 128, 129] unnormalized
        for i, p in enumerate(pairs):
            acc = r[i].reshape(S, F + 1)
            out[p] = acc[:, :F] / acc[:, F:F + 1] + bv[p % H]
    full = out.reshape(B, H, S, F)
    if trace:
        return full, res
    return full


# revision 8
# speedup vs baseline: 1.0485x; 1.0425x over previous
"""Causal multi-head attention (B=2, H=16, S=2048, F=128) on 8 TRN2 NeuronCores.

Sharding: tensor-parallel over the (batch, head) axis — 32 independent
(b, h) attention problems, 4 per core. No collectives needed.

Score algebra (per head): with M = Wq^T Wk, u = Wq^T bk, w = Wk^T bq,
  s[q,k] = (x_q Wq^T + bq)·(x_k Wk^T + bk)
         = x_q·(M x_k + u) + w·x_k + const
and the per-head const drops inside softmax.  So instead of projecting
Q and K separately (two matmul passes), the kernel computes a single
z = M^T-stationary projection (z_k = M x_k + u), and the per-key scalar
beta_k = w·x_k rides along the V projection as an extra moving column.
beta is applied inside the exp via the ACT per-partition bias operand.

Per-head on-chip algorithm (no on-chip transposes):
  - host pre-transposes x to xT [F, S] and supplies mt = M^T [f,f'],
    wvx = [Wv^T | 0 | SCALE*w] [f, F+2].
  - ZT = mt.T @ xT (PSUM) + u -> SBUF [f', s]   (one pass, not two)
  - V' = xT_tile.T @ wvx per s-tile -> [s, F+2]: cols 0..127 V, col 128
    becomes the softmax-denominator ones column (bias add), col 129 is
    SCALE*beta for that key tile.
  - For each k-tile: ST strip = Z_tile . X -> [k, q<=1024] in PSUM,
    exp(SCALE*st + SCALE*beta) on ACT -> PT bf16 in SBUF, triangle mask
    on the diagonal block, then AV: acc[q, 129] += PT_tile.T @ V'_tile
    accumulated over k-tiles in PSUM.  Column 128 = sum(exp).
  - Finished accumulator groups are DMA'd straight from PSUM to DRAM
    UNNORMALIZED; the host divides by the denominator column and adds
    bv (out = acc/den + bv, exact because sum_k P·bv = den·bv).
"""

import math

import numpy as np
import ml_dtypes

import concourse.bass as bass
import concourse.tile as tile
import concourse.mybir as mybir
from concourse import bacc, bass_utils

B, H, S, F = 2, 16, 2048, 128
NCORES = 8
HPC = (B * H) // NCORES  # (b,h) pairs per core
SCALE = 1.0 / math.sqrt(F)
HALF = S // 2  # q processed in two 1024-wide halves (PSUM budget)
NKT = S // F   # 16 k-tiles per head
GSTRIDE = 136  # col stride of packed PSUM groups (32B aligned)
VW = F + 2     # V' projection width: [Wv | denom-ones | beta]

_cache = {}


def _build():
    f32 = mybir.dt.float32
    bf16 = mybir.dt.bfloat16
    Exp = mybir.ActivationFunctionType.Exp
    Ident = mybir.ActivationFunctionType.Identity

    nc = bacc.Bacc("TRN2")

    xtb = nc.dram_tensor("xtbh", [HPC, F, S], bf16, kind="ExternalInput")
    mtd = nc.dram_tensor("mtd", [HPC, F, F], bf16, kind="ExternalInput")
    wvx = nc.dram_tensor("wvx", [HPC, F, VW], bf16, kind="ExternalInput")
    ud = nc.dram_tensor("ud", [F, HPC], f32, kind="ExternalInput")
    vb1 = nc.dram_tensor("vb1", [1, GSTRIDE + VW], bf16, kind="ExternalInput")
    msk = nc.dram_tensor("msk", [F, F], bf16, kind="ExternalInput")
    out = nc.dram_tensor("out", [HPC, NKT, F, F + 1], f32,
                         kind="ExternalOutput")

    with tile.TileContext(nc) as tc, \
            tc.tile_pool(name="consts", bufs=1) as consts, \
            tc.tile_pool(name="xin", bufs=3) as xin, \
            tc.tile_pool(name="zt", bufs=3) as ztp, \
            tc.tile_pool(name="vp", bufs=2 * NKT) as vpp, \
            tc.tile_pool(name="pt", bufs=9) as ptp, \
            tc.tile_pool(name="outs", bufs=4) as outp, \
            tc.tile_pool(name="st", bufs=2, space="PSUM") as stp, \
            tc.tile_pool(name="av", bufs=3, space="PSUM") as avp, \
            tc.tile_pool(name="vq", bufs=1, space="PSUM") as vqp:

        c_u = consts.tile([F, HPC], f32, tag="u")
        nc.gpsimd.dma_start(out=c_u, in_=ud[:, :])
        c_mask = consts.tile([F, F], bf16, tag="msk")
        nc.gpsimd.dma_start(out=c_mask, in_=msk[:, :])
        # V'-evac bias, broadcast to all partitions: 1.0 at the two
        # denominator columns (128 and GSTRIDE+128), 0 elsewhere
        c_vb = consts.tile([128, GSTRIDE + VW], bf16, tag="vb")
        vbr = vb1[0]
        nc.gpsimd.dma_start(
            out=c_vb,
            in_=bass.AP(tensor=vbr.tensor, offset=vbr.offset,
                        ap=[[0, 128]] + list(vbr.ap)))

        # deferred AV-batch emission: by the time an AV batch is
        # emitted, the exp it reads finished ~2 iterations ago, so the
        # PE never stalls waiting on ACT
        SKEW = 5
        pending = []

        def flush_pending(keep=0):
            while len(pending) > keep:
                pending.pop(0)()

        def make_prelude(hd):
            """Emission closures for head hd's input DMAs, Z and V'
            projections. Popped one-per-ki during head hd-1's k-loop so
            this work hides under the previous head's softmax."""
            st8 = {"vav": [], "vbeta": []}

            def dmas(hd=hd):
                # quarters, ordered so the first z chunk / strip / V'
                # tile can start after ~1/4 of the x load
                mt = xin.tile([F, F], bf16, tag="mt", name=f"mt_{hd}")
                nc.sync.dma_start(out=mt, in_=mtd[hd])
                xbh = xin.tile([F, S], bf16, tag="xbh", name=f"xbh_{hd}")
                nc.sync.dma_start(out=xbh[:, 0:512], in_=xtb[hd][:, 0:512])
                wv = xin.tile([F, VW], bf16, tag="wv", name=f"wv_{hd}")
                nc.sync.dma_start(out=wv, in_=wvx[hd])
                for qtr in range(1, 4):
                    nc.sync.dma_start(
                        out=xbh[:, 512 * qtr:512 * (qtr + 1)],
                        in_=xtb[hd][:, 512 * qtr:512 * (qtr + 1)])
                st8["xbh"], st8["mt"], st8["wv"] = xbh, mt, wv
                st8["zt"] = ztp.tile([F, S], bf16, tag="zt", name=f"zt_{hd}")

            def z_chunk(c, hd=hd, pool=None, tag="vq", act=False):
                ps = (pool or vqp).tile([128, 512], f32, tag=tag,
                                        name=f"z_{hd}_{c}")
                nc.tensor.matmul(
                    ps[:, 0:512], st8["mt"][:, :],
                    st8["xbh"][:, 512 * c:512 * (c + 1)],
                    start=True, stop=True)
                dst = st8["zt"][:, 512 * c:512 * (c + 1)]
                if act:  # startup only: ACT is idle then
                    nc.scalar.activation(out=dst, in_=ps[:, 0:512],
                                         func=Ident, bias=c_u[:, hd:hd + 1])
                else:
                    nc.vector.tensor_scalar_add(dst, ps[:, 0:512],
                                                c_u[:, hd:hd + 1])

            def vpd_tile(j, hd=hd):
                # two s-tiles of V' share one PSUM bank (cols 0 and
                # GSTRIDE) and one SBUF tile + one evacuation copy. The
                # second prefill's start=True clears the whole bank's
                # has_written, but pair A is fully written by then
                # (data persists).
                ps = vqp.tile([128, 512], f32, tag="vq",
                              name=f"vps_{hd}_{j}")
                for half_j in range(2):
                    si = 2 * j + half_j
                    g = GSTRIDE * half_j
                    nc.tensor.matmul(
                        ps[:, g:g + VW],
                        st8["xbh"][:, 128 * si:128 * (si + 1)],
                        st8["wv"][:, :],
                        start=True, stop=True, skip_group_check=True)
                vt = vpp.tile([128, GSTRIDE + VW], bf16, tag="vp",
                              name=f"vp_{hd}_{j}")
                # evacuation copy with the denominator-ones column
                # folded in via the broadcast bias tile
                nc.vector.scalar_tensor_tensor(
                    out=vt[:, :], in0=ps[:, 0:GSTRIDE + VW], scalar=1.0,
                    in1=c_vb[:, :], op0=mybir.AluOpType.mult,
                    op1=mybir.AluOpType.add)
                for half_j in range(2):
                    g = GSTRIDE * half_j
                    st8["vav"].append(vt[:, g:g + F + 1])
                    st8["vbeta"].append(vt[:, g + F + 1:g + F + 2])

            # ordered so V' pairs arrive ahead of the exps that read
            # their beta column, and Z chunks ahead of the strips that
            # read them; 13 closures <= 24 k-iterations
            closures = [dmas]
            if hd == 0:
                # startup: spread the first chunks over idle PSUM pools
                # and both ACT+DVE so they run concurrently
                closures.append(lambda: z_chunk(0, pool=stp, tag="st",
                                                act=True))
                closures.append(lambda: z_chunk(1, pool=avp, tag="av"))
                closures.append(lambda: vpd_tile(0))
                order = [lambda: z_chunk(2), lambda: vpd_tile(1),
                         lambda: z_chunk(3), lambda: vpd_tile(2),
                         lambda: vpd_tile(3), lambda: vpd_tile(4),
                         lambda: vpd_tile(5), lambda: vpd_tile(6),
                         lambda: vpd_tile(7)]
            else:
                closures.append(lambda: z_chunk(0))
                closures.append(lambda: z_chunk(1))
                closures.append(lambda: vpd_tile(0))
                order = [lambda: z_chunk(2), lambda: vpd_tile(1),
                         lambda: z_chunk(3), lambda: vpd_tile(2),
                         lambda: vpd_tile(3), lambda: vpd_tile(4),
                         lambda: vpd_tile(5), lambda: vpd_tile(6),
                         lambda: vpd_tile(7)]
            closures.extend(order)
            return st8, closures

        head_state = {}
        head_state[0], prelude = make_prelude(0)
        for _ in range(4):  # dmas + z0/z1 on parallel PSUM slots + vpd0
            prelude.pop(0)()

        for hd in range(HPC):
            if hd > 0:
                while prelude:  # leftovers from the previous k-loop
                    prelude.pop(0)()
            if hd + 1 < HPC:
                head_state[hd + 1], nxt = make_prelude(hd + 1)
                prelude.extend(nxt)
            zt_t = head_state[hd]["zt"]
            xbh_t = head_state[hd]["xbh"]
            vav = head_state[hd]["vav"]
            vbeta = head_state[hd]["vbeta"]

            # --- attention, q in two 1024-wide halves ---
            for half in range(2):
                q0 = half * HALF
                nk = (half + 1) * (HALF // 128)  # k-tiles for this half
                hstate = {}

                for ki in range(nk):
                    ks = 128 * ki
                    ls = max(0, ks - q0)  # local start col within strip
                    strip = stp.tile([128, 1024], f32, tag="st")
                    bounds = [ls, 512, 1024] if ls < 512 else [ls, 1024]
                    pieces = list(zip(bounds[:-1], bounds[1:]))
                    # first ST piece, then the deferred AV batch (whose
                    # matmul stream hides this piece's drain), then the
                    # second piece (weight load hides under AV drains)
                    nc.tensor.matmul(
                        strip[:, pieces[0][0]:pieces[0][1]],
                        zt_t[:, ks:ks + 128],
                        xbh_t[:, q0 + pieces[0][0]:q0 + pieces[0][1]],
                        start=True, stop=True)
                    if prelude:  # hide next head's Z/V' here
                        prelude.pop(0)()
                    flush_pending(keep=SKEW - 1)
                    for c0, c1 in pieces[1:]:
                        nc.tensor.matmul(
                            strip[:, c0:c1], zt_t[:, ks:ks + 128],
                            xbh_t[:, q0 + c0:q0 + c1],
                            start=True, stop=True)
                    ptile = ptp.tile([128, 1024], bf16, tag="pt")
                    nc.scalar.activation(
                        out=ptile[:, ls:1024], in_=strip[:, ls:1024],
                        func=Exp, scale=SCALE, bias=vbeta[ki])
                    if ks >= q0:  # zero below-diagonal of the diag block
                        nc.vector.tensor_mul(
                            ptile[:, ls:ls + 128], ptile[:, ls:ls + 128],
                            c_mask[:, :])

                    def av_batch(hd=hd, half=half, ki=ki, ptile=ptile,
                                 hstate=hstate, vav=vav):
                        if ki == 0:
                            # start=True clears has_written for the
                            # WHOLE bank (per partition), so only the
                            # FIRST matmul into each bank (qt%3==0 at
                            # ki=0) may carry it; the other packed
                            # groups' first writes find their bits
                            # clear and overwrite.
                            hstate["avts"] = [
                                avp.tile([128, 512], f32, tag="av",
                                         name=f"avacc_{hd}_{half}_{i}")
                                for i in range(3)]
                        avts = hstate["avts"]
                        for qt in range(max(0, ki - 8 * half), 8):
                            qg = 8 * half + qt
                            g = GSTRIDE * (qt % 3)
                            acc = avts[qt // 3][:, g:g + F + 1]
                            nc.tensor.matmul(
                                acc, ptile[:, 128 * qt:128 * qt + 128],
                                vav[ki][:, :],
                                start=(ki == 0 and qt % 3 == 0),
                                stop=(ki == qg),
                                skip_group_check=True)
                        # once a whole accumulator bank is finished,
                        # stage it to SBUF with ONE copy (DMA cannot
                        # read PSUM) and DMA the (unnormalized) groups
                        # out; host divides by the denominator column
                        for bank in range(3):
                            last_qt = min(3 * bank + 2, 7)
                            if ki != 8 * half + last_qt:
                                continue
                            ng = last_qt - 3 * bank + 1
                            w = GSTRIDE * (ng - 1) + F + 1
                            stage = outp.tile([128, 2 * GSTRIDE + F + 1],
                                              f32, tag="ot")
                            nc.vector.tensor_copy(
                                out=stage[:, 0:w],
                                in_=avts[bank][:, 0:w])
                            for qt in range(3 * bank, last_qt + 1):
                                qg = 8 * half + qt
                                g = GSTRIDE * (qt % 3)
                                # alternate queues so the drain at the
                                # end of the kernel is parallel
                                eng = nc.gpsimd if (qt & 1) else nc.sync
                                eng.dma_start(
                                    out=out[hd, qg],
                                    in_=stage[:, g:g + F + 1])

                    pending.append(av_batch)
        flush_pending()

    nc.compile()
    return nc


def _prep_inputs(x, Wq, Wk, Wv, bq, bk, bv):
    """Shard + pre-transpose + fold weights on host. 8 core in_maps."""
    bf16 = ml_dtypes.bfloat16
    xf = np.ascontiguousarray(
        x.reshape(B * H, S, F).transpose(0, 2, 1))          # [32, F, S]
    xfb = xf.astype(bf16)
    # mt = M^T = (Wq^T Wk)^T = Wk^T Wq, per head  [f, f']
    mt = np.einsum("hef,heg->hfg", Wk, Wq).astype(bf16)     # [H, f, g=f']
    u = np.einsum("hef,he->hf", Wq, bk).astype(np.float32)  # [H, f']
    w = np.einsum("hef,he->hf", Wk, bq).astype(np.float32)  # [H, f]
    # wvx = [Wv^T | 0 | SCALE*w]  [f, VW]
    wvxh = np.zeros((H, F, VW), np.float32)
    wvxh[:, :, :F] = Wv.transpose(0, 2, 1)
    wvxh[:, :, F + 1] = SCALE * w
    wvxh = wvxh.astype(bf16)
    vb = np.zeros((1, GSTRIDE + VW), np.float32)
    vb[0, F] = 1.0
    vb[0, GSTRIDE + F] = 1.0
    mask = np.triu(np.ones((F, F), np.float32)).astype(bf16)  # keep r <= c

    in_maps = []
    for c in range(NCORES):
        pairs = list(range(HPC * c, HPC * (c + 1)))
        heads = [p % H for p in pairs]
        m = {
            "xtbh": np.ascontiguousarray(xfb[pairs]),
            "mtd": np.ascontiguousarray(mt[heads]),
            "wvx": np.ascontiguousarray(wvxh[heads]),
            "ud": np.ascontiguousarray(u[heads].T).astype(np.float32),
            "vb1": vb.astype(bf16),
            "msk": mask,
        }
        in_maps.append(m)
    return in_maps


def kernel(x, Wq, Wk, Wv, bq, bk, bv, trace=False):
    x, Wq, Wk, Wv = (np.asarray(a, np.float32) for a in (x, Wq, Wk, Wv))
    bq, bk, bv = (np.asarray(a, np.float32) for a in (bq, bk, bv))

    if "nc" not in _cache:
        _cache["nc"] = _build()
    nc = _cache["nc"]

    in_maps = _prep_inputs(x, Wq, Wk, Wv, bq, bk, bv)
    res = bass_utils.run_bass_kernel_spmd(
        nc, in_maps, core_ids=list(range(NCORES)), trace=trace)

    out = np.empty((B * H, S, F), np.float32)
    for c in range(NCORES):
        pairs = range(HPC * c, HPC * (c + 1))
        r = res.results[c]["out"]  # [HPC, NKT, 128, 129] unnormalized
        for i, p in enumerate(pairs):
            acc = r[i].reshape(S, F + 1)
            out[p] = acc[:, :F] / acc[:, F:F + 1] + bv[p % H]
    full = out.reshape(B, H, S, F)
    if trace:
        return full, res
    return full


# revision 17
# speedup vs baseline: 1.0510x; 1.0023x over previous
"""Causal multi-head attention (B=2, H=16, S=2048, F=128) on 8 TRN2 NeuronCores.

Sharding: tensor-parallel over the (batch, head) axis — 32 independent
(b, h) attention problems, 4 per core. No collectives needed.

Score algebra (per head): with M = Wq^T Wk, u = Wq^T bk, w = Wk^T bq,
  s[q,k] = (x_q Wq^T + bq)·(x_k Wk^T + bk)
         = x_q·(M x_k + u) + w·x_k + const
and the per-head const drops inside softmax.  So instead of projecting
Q and K separately (two matmul passes), the kernel computes a single
z = M^T-stationary projection (z_k = M x_k + u), and the per-key scalar
beta_k = w·x_k rides along the V projection as an extra moving column.
beta is applied inside the exp via the ACT per-partition bias operand.

Per-head on-chip algorithm (no on-chip transposes):
  - host pre-transposes x to xT [F, S] and supplies mt = M^T [f,f'],
    wvx = [Wv^T | 0 | SCALE*w] [f, F+2].
  - ZT = mt.T @ xT (PSUM) + u -> SBUF [f', s]   (one pass, not two)
  - V' = xT_tile.T @ wvx per s-tile -> [s, F+2]: cols 0..127 V, col 128
    becomes the softmax-denominator ones column (bias add), col 129 is
    SCALE*beta for that key tile.
  - For each k-tile: ST strip = Z_tile . X -> [k, q<=1024] in PSUM,
    exp(SCALE*st + SCALE*beta) on ACT -> PT bf16 in SBUF, triangle mask
    on the diagonal block, then AV: acc[q, 129] += PT_tile.T @ V'_tile
    accumulated over k-tiles in PSUM.  Column 128 = sum(exp).
  - Finished accumulator groups are DMA'd straight from PSUM to DRAM
    UNNORMALIZED; the host divides by the denominator column and adds
    bv (out = acc/den + bv, exact because sum_k P·bv = den·bv).
"""

import math

import numpy as np
import ml_dtypes

import concourse.bass as bass
import concourse.tile as tile
import concourse.mybir as mybir
from concourse import bacc, bass_utils

B, H, S, F = 2, 16, 2048, 128
NCORES = 8
HPC = (B * H) // NCORES  # (b,h) pairs per core
SCALE = 1.0 / math.sqrt(F)
HALF = S // 2  # q processed in two 1024-wide halves (PSUM budget)
NKT = S // F   # 16 k-tiles per head
GSTRIDE = 136  # col stride of packed PSUM groups (32B aligned)
VW = F + 2     # V' projection width: [Wv | denom-ones | beta]

_cache = {}


def _build():
    f32 = mybir.dt.float32
    bf16 = mybir.dt.bfloat16
    Exp = mybir.ActivationFunctionType.Exp
    Ident = mybir.ActivationFunctionType.Identity

    nc = bacc.Bacc("TRN2")

    xtb = nc.dram_tensor("xtbh", [HPC, F, S], bf16, kind="ExternalInput")
    mtd = nc.dram_tensor("mtd", [HPC, F, F], bf16, kind="ExternalInput")
    wvx = nc.dram_tensor("wvx", [HPC, F, VW], bf16, kind="ExternalInput")
    ud = nc.dram_tensor("ud", [F, HPC], f32, kind="ExternalInput")
    vb1 = nc.dram_tensor("vb1", [128, GSTRIDE + VW], bf16,
                         kind="ExternalInput")
    msk = nc.dram_tensor("msk", [F, F], bf16, kind="ExternalInput")
    out = nc.dram_tensor("out", [HPC, NKT, F, F + 1], f32,
                         kind="ExternalOutput")

    with tile.TileContext(nc) as tc, \
            tc.tile_pool(name="consts", bufs=1) as consts, \
            tc.tile_pool(name="xin", bufs=3) as xin, \
            tc.tile_pool(name="zt", bufs=3) as ztp, \
            tc.tile_pool(name="vp", bufs=2 * NKT) as vpp, \
            tc.tile_pool(name="pt", bufs=9) as ptp, \
            tc.tile_pool(name="outs", bufs=4) as outp, \
            tc.tile_pool(name="st", bufs=2, space="PSUM") as stp, \
            tc.tile_pool(name="av", bufs=3, space="PSUM") as avp, \
            tc.tile_pool(name="vq", bufs=1, space="PSUM") as vqp:

        c_u = consts.tile([F, HPC], f32, tag="u")
        nc.gpsimd.dma_start(out=c_u, in_=ud[:, :])
        c_mask = consts.tile([F, F], bf16, tag="msk")
        nc.gpsimd.dma_start(out=c_mask, in_=msk[:, :])
        # V'-evac bias (host-materialized across partitions): 1.0 at the
        # two denominator columns (128 and GSTRIDE+128), 0 elsewhere
        c_vb = consts.tile([128, GSTRIDE + VW], bf16, tag="vb")
        nc.gpsimd.dma_start(out=c_vb, in_=vb1[:, :])
        # touch Exp once so ACT's table set loads during the input DMAs
        # instead of on the first real softmax strip
        warm = consts.tile([1, 8], f32, tag="warm")
        nc.vector.memset(warm[:, 0:8], 0.0)
        nc.scalar.activation(out=warm[:, 0:8], in_=warm[:, 0:8],
                             func=Exp)

        # deferred AV-batch emission: by the time an AV batch is
        # emitted, the exp it reads finished ~2 iterations ago, so the
        # PE never stalls waiting on ACT
        SKEW = 5
        pending = []

        def flush_pending(keep=0):
            while len(pending) > keep:
                pending.pop(0)()

        def make_prelude(hd):
            """Emission closures for head hd's input DMAs, Z and V'
            projections. Popped one-per-ki during head hd-1's k-loop so
            this work hides under the previous head's softmax."""
            st8 = {"vav": [], "vbeta": []}

            def dmas(hd=hd):
                # quarters, ordered so the first z chunk / strip / V'
                # tile can start after ~1/4 of the x load
                mt = xin.tile([F, F], bf16, tag="mt", name=f"mt_{hd}")
                nc.sync.dma_start(out=mt, in_=mtd[hd])
                xbh = xin.tile([F, S], bf16, tag="xbh", name=f"xbh_{hd}")
                nc.sync.dma_start(out=xbh[:, 0:512], in_=xtb[hd][:, 0:512])
                wv = xin.tile([F, VW], bf16, tag="wv", name=f"wv_{hd}")
                nc.sync.dma_start(out=wv, in_=wvx[hd])
                for qtr in range(1, 4):
                    nc.sync.dma_start(
                        out=xbh[:, 512 * qtr:512 * (qtr + 1)],
                        in_=xtb[hd][:, 512 * qtr:512 * (qtr + 1)])
                st8["xbh"], st8["mt"], st8["wv"] = xbh, mt, wv
                st8["zt"] = ztp.tile([F, S], bf16, tag="zt", name=f"zt_{hd}")

            def z_chunk(c, hd=hd, pool=None, tag="vq", act=False):
                ps = (pool or vqp).tile([128, 512], f32, tag=tag,
                                        name=f"z_{hd}_{c}")
                nc.tensor.matmul(
                    ps[:, 0:512], st8["mt"][:, :],
                    st8["xbh"][:, 512 * c:512 * (c + 1)],
                    start=True, stop=True)
                dst = st8["zt"][:, 512 * c:512 * (c + 1)]
                if act:  # startup only: ACT is idle then
                    nc.scalar.activation(out=dst, in_=ps[:, 0:512],
                                         func=Ident, bias=c_u[:, hd:hd + 1])
                else:
                    nc.vector.tensor_scalar_add(dst, ps[:, 0:512],
                                                c_u[:, hd:hd + 1])

            def vpd_tile(j, hd=hd):
                # two s-tiles of V' share one PSUM bank (cols 0 and
                # GSTRIDE) and one SBUF tile + one evacuation copy. The
                # second prefill's start=True clears the whole bank's
                # has_written, but pair A is fully written by then
                # (data persists).
                ps = vqp.tile([128, 512], f32, tag="vq",
                              name=f"vps_{hd}_{j}")
                for half_j in range(2):
                    si = 2 * j + half_j
                    g = GSTRIDE * half_j
                    nc.tensor.matmul(
                        ps[:, g:g + VW],
                        st8["xbh"][:, 128 * si:128 * (si + 1)],
                        st8["wv"][:, :],
                        start=True, stop=True, skip_group_check=True)
                vt = vpp.tile([128, GSTRIDE + VW], bf16, tag="vp",
                              name=f"vp_{hd}_{j}")
                # evacuation copy with the denominator-ones column
                # folded in via the broadcast bias tile
                nc.vector.scalar_tensor_tensor(
                    out=vt[:, :], in0=ps[:, 0:GSTRIDE + VW], scalar=1.0,
                    in1=c_vb[:, :], op0=mybir.AluOpType.mult,
                    op1=mybir.AluOpType.add)
                for half_j in range(2):
                    g = GSTRIDE * half_j
                    st8["vav"].append(vt[:, g:g + F + 1])
                    st8["vbeta"].append(vt[:, g + F + 1:g + F + 2])

            # ordered so V' pairs arrive ahead of the exps that read
            # their beta column, and Z chunks ahead of the strips that
            # read them; 13 closures <= 24 k-iterations
            closures = [dmas]
            if hd == 0:
                # startup: half0 strips 0-3 need only z chunk 0 and x
                # quarters 0-1; order pops so nothing head-of-line
                # blocks the PE queue on a not-yet-arrived x quarter
                closures.append(lambda: z_chunk(0, pool=stp, tag="st",
                                                act=True))
                closures.append(lambda: vpd_tile(0))
                order = [lambda: vpd_tile(1), lambda: z_chunk(1),
                         lambda: vpd_tile(2), lambda: vpd_tile(3),
                         lambda: z_chunk(2), lambda: z_chunk(3),
                         lambda: vpd_tile(4), lambda: vpd_tile(5),
                         lambda: vpd_tile(6), lambda: vpd_tile(7)]
            else:
                closures.append(lambda: z_chunk(0))
                closures.append(lambda: z_chunk(1))
                closures.append(lambda: vpd_tile(0))
                order = [lambda: z_chunk(2), lambda: vpd_tile(1),
                         lambda: z_chunk(3), lambda: vpd_tile(2),
                         lambda: vpd_tile(3), lambda: vpd_tile(4),
                         lambda: vpd_tile(5), lambda: vpd_tile(6),
                         lambda: vpd_tile(7)]
            closures.extend(order)
            return st8, closures

        head_state = {}
        head_state[0], prelude = make_prelude(0)
        for _ in range(3):  # dmas + z0 + vpd0; rest pops in the k-loop
            prelude.pop(0)()
        total_iters = HPC * 24  # for the end-of-kernel pending drain
        it = 0

        for hd in range(HPC):
            if hd > 0:
                while prelude:  # leftovers from the previous k-loop
                    prelude.pop(0)()
            if hd + 1 < HPC:
                head_state[hd + 1], nxt = make_prelude(hd + 1)
                prelude.extend(nxt)
            zt_t = head_state[hd]["zt"]
            xbh_t = head_state[hd]["xbh"]
            vav = head_state[hd]["vav"]
            vbeta = head_state[hd]["vbeta"]

            # --- attention, q in two 1024-wide halves ---
            for half in range(2):
                q0 = half * HALF
                nk = (half + 1) * (HALF // 128)  # k-tiles for this half
                hstate = {}

                for ki in range(nk):
                    ks = 128 * ki
                    ls = max(0, ks - q0)  # local start col within strip
                    strip = stp.tile([128, 1024], f32, tag="st")
                    bounds = [ls, 512, 1024] if ls < 512 else [ls, 1024]
                    pieces = list(zip(bounds[:-1], bounds[1:]))
                    # both ST pieces first so exp can start as early as
                    # possible; the deferred AV batch then streams on
                    # the PE while ACT runs this strip's exp
                    for c0, c1 in pieces:
                        nc.tensor.matmul(
                            strip[:, c0:c1], zt_t[:, ks:ks + 128],
                            xbh_t[:, q0 + c0:q0 + c1],
                            start=True, stop=True)
                    ptile = ptp.tile([128, 1024], bf16, tag="pt")
                    nc.scalar.activation(
                        out=ptile[:, ls:1024], in_=strip[:, ls:1024],
                        func=Exp, scale=SCALE, bias=vbeta[ki])
                    if ks >= q0:  # zero below-diagonal of the diag block
                        nc.vector.tensor_mul(
                            ptile[:, ls:ls + 128], ptile[:, ls:ls + 128],
                            c_mask[:, :])
                    if prelude:  # hide next head's Z/V' here
                        prelude.pop(0)()
                    it += 1
                    keep = min(SKEW - 1, total_iters - it)
                    flush_pending(keep=keep)

                    def av_batch(hd=hd, half=half, ki=ki, ptile=ptile,
                                 hstate=hstate, vav=vav):
                        if ki == 0:
                            # start=True clears has_written for the
                            # WHOLE bank (per partition), so only the
                            # FIRST matmul into each bank (qt%3==0 at
                            # ki=0) may carry it; the other packed
                            # groups' first writes find their bits
                            # clear and overwrite.
                            hstate["avts"] = [
                                avp.tile([128, 512], f32, tag="av",
                                         name=f"avacc_{hd}_{half}_{i}")
                                for i in range(3)]
                        avts = hstate["avts"]
                        for qt in range(max(0, ki - 8 * half), 8):
                            qg = 8 * half + qt
                            g = GSTRIDE * (qt % 3)
                            acc = avts[qt // 3][:, g:g + F + 1]
                            nc.tensor.matmul(
                                acc, ptile[:, 128 * qt:128 * qt + 128],
                                vav[ki][:, :],
                                start=(ki == 0 and qt % 3 == 0),
                                stop=(ki == qg),
                                skip_group_check=True)
                        # once a whole accumulator bank is finished,
                        # stage it to SBUF with ONE copy (DMA cannot
                        # read PSUM) and DMA the (unnormalized) groups
                        # out; host divides by the denominator column
                        for bank in range(3):
                            last_qt = min(3 * bank + 2, 7)
                            if ki != 8 * half + last_qt:
                                continue
                            ng = last_qt - 3 * bank + 1
                            w = GSTRIDE * (ng - 1) + F + 1
                            stage = outp.tile([128, 2 * GSTRIDE + F + 1],
                                              f32, tag="ot")
                            nc.vector.tensor_copy(
                                out=stage[:, 0:w],
                                in_=avts[bank][:, 0:w])
                            for qt in range(3 * bank, last_qt + 1):
                                qg = 8 * half + qt
                                g = GSTRIDE * (qt % 3)
                                # alternate queues so the drain at the
                                # end of the kernel is parallel
                                eng = nc.gpsimd if (qt & 1) else nc.sync
                                eng.dma_start(
                                    out=out[hd, qg],
                                    in_=stage[:, g:g + F + 1])

                    pending.append(av_batch)
        flush_pending()

    nc.compile()
    return nc


def _prep_inputs(x, Wq, Wk, Wv, bq, bk, bv):
    """Shard + pre-transpose + fold weights on host. 8 core in_maps."""
    bf16 = ml_dtypes.bfloat16
    xf = np.ascontiguousarray(
        x.reshape(B * H, S, F).transpose(0, 2, 1))          # [32, F, S]
    xfb = xf.astype(bf16)
    # mt = M^T = (Wq^T Wk)^T = Wk^T Wq, per head  [f, f']
    mt = np.einsum("hef,heg->hfg", Wk, Wq).astype(bf16)     # [H, f, g=f']
    u = np.einsum("hef,he->hf", Wq, bk).astype(np.float32)  # [H, f']
    w = np.einsum("hef,he->hf", Wk, bq).astype(np.float32)  # [H, f]
    # wvx = [Wv^T | 0 | SCALE*w]  [f, VW]
    wvxh = np.zeros((H, F, VW), np.float32)
    wvxh[:, :, :F] = Wv.transpose(0, 2, 1)
    wvxh[:, :, F + 1] = SCALE * w
    wvxh = wvxh.astype(bf16)
    vb = np.zeros((128, GSTRIDE + VW), np.float32)
    vb[:, F] = 1.0
    vb[:, GSTRIDE + F] = 1.0
    mask = np.triu(np.ones((F, F), np.float32)).astype(bf16)  # keep r <= c

    in_maps = []
    for c in range(NCORES):
        pairs = list(range(HPC * c, HPC * (c + 1)))
        heads = [p % H for p in pairs]
        m = {
            "xtbh": np.ascontiguousarray(xfb[pairs]),
            "mtd": np.ascontiguousarray(mt[heads]),
            "wvx": np.ascontiguousarray(wvxh[heads]),
            "ud": np.ascontiguousarray(u[heads].T).astype(np.float32),
            "vb1": vb.astype(bf16),
            "msk": mask,
        }
        in_maps.append(m)
    return in_maps


def kernel(x, Wq, Wk, Wv, bq, bk, bv, trace=False):
    x, Wq, Wk, Wv = (np.asarray(a, np.float32) for a in (x, Wq, Wk, Wv))
    bq, bk, bv = (np.asarray(a, np.float32) for a in (bq, bk, bv))

    if "nc" not in _cache:
        _cache["nc"] = _build()
    nc = _cache["nc"]

    in_maps = _prep_inputs(x, Wq, Wk, Wv, bq, bk, bv)
    res = bass_utils.run_bass_kernel_spmd(
        nc, in_maps, core_ids=list(range(NCORES)), trace=trace)

    out = np.empty((B * H, S, F), np.float32)
    for c in range(NCORES):
        pairs = range(HPC * c, HPC * (c + 1))
        r = res.results[c]["out"]  # [HPC, NKT, 128, 129] unnormalized
        for i, p in enumerate(pairs):
            acc = r[i].reshape(S, F + 1)
            out[p] = acc[:, :F] / acc[:, F:F + 1] + bv[p % H]
    full = out.reshape(B, H, S, F)
    if trace:
        return full, res
    return full


# revision 22
# speedup vs baseline: 1.0691x; 1.0173x over previous
"""Causal multi-head attention (B=2, H=16, S=2048, F=128) on 8 TRN2 NeuronCores.

Sharding: tensor-parallel over the (batch, head) axis — 32 independent
(b, h) attention problems, 4 per core. No collectives needed.

Score algebra (per head): with M = Wq^T Wk, u = Wq^T bk, w = Wk^T bq,
  s[q,k] = (x_q Wq^T + bq)·(x_k Wk^T + bk)
         = x_q·(M x_k + u) + w·x_k + const
and the per-head const drops inside softmax.  So instead of projecting
Q and K separately (two matmul passes), the kernel computes a single
z = M^T-stationary projection (z_k = M x_k + u), and the per-key scalar
beta_k = w·x_k rides along the V projection as an extra moving column.
beta is applied inside the exp via the ACT per-partition bias operand.

Per-head on-chip algorithm (no on-chip transposes):
  - host pre-transposes x to xT [F, S] and supplies mt = M^T [f,f'],
    wvx = [Wv^T | 0 | SCALE*w] [f, F+2].
  - ZT = mt.T @ xT (PSUM) + u -> SBUF [f', s]   (one pass, not two)
  - V' = xT_tile.T @ wvx per s-tile -> [s, F+2]: cols 0..127 V, col 128
    becomes the softmax-denominator ones column (bias add), col 129 is
    SCALE*beta for that key tile.
  - For each k-tile: ST strip = Z_tile . X -> [k, q<=1024] in PSUM,
    exp(SCALE*st + SCALE*beta) on ACT -> PT bf16 in SBUF, triangle mask
    on the diagonal block, then AV: acc[q, 129] += PT_tile.T @ V'_tile
    accumulated over k-tiles in PSUM.  Column 128 = sum(exp).
  - Finished accumulator groups are DMA'd straight from PSUM to DRAM
    UNNORMALIZED; the host divides by the denominator column and adds
    bv (out = acc/den + bv, exact because sum_k P·bv = den·bv).
"""

import math

import numpy as np
import ml_dtypes

import concourse.bass as bass
import concourse.tile as tile
import concourse.mybir as mybir
from concourse import bacc, bass_utils

B, H, S, F = 2, 16, 2048, 128
NCORES = 8
HPC = (B * H) // NCORES  # (b,h) pairs per core
SCALE = 1.0 / math.sqrt(F)
HALF = S // 2  # q processed in two 1024-wide halves (PSUM budget)
NKT = S // F   # 16 k-tiles per head
GSTRIDE = 136  # col stride of packed PSUM groups (32B aligned)
VW = F + 2     # V' projection width: [Wv | denom-ones | beta]

_cache = {}


def _build():
    f32 = mybir.dt.float32
    bf16 = mybir.dt.bfloat16
    Exp = mybir.ActivationFunctionType.Exp
    Ident = mybir.ActivationFunctionType.Identity

    nc = bacc.Bacc("TRN2")

    xtb = nc.dram_tensor("xtbh", [HPC, F, S], bf16, kind="ExternalInput")
    mtd = nc.dram_tensor("mtd", [HPC, F, F], bf16, kind="ExternalInput")
    wvx = nc.dram_tensor("wvx", [HPC, F, VW], bf16, kind="ExternalInput")
    ud = nc.dram_tensor("ud", [F, HPC], f32, kind="ExternalInput")
    vb1 = nc.dram_tensor("vb1", [128, GSTRIDE + VW], bf16,
                         kind="ExternalInput")
    msk = nc.dram_tensor("msk", [F, F], bf16, kind="ExternalInput")
    out = nc.dram_tensor("out", [HPC, NKT, F, F + 1], f32,
                         kind="ExternalOutput")

    with tile.TileContext(nc) as tc, \
            tc.tile_pool(name="consts", bufs=1) as consts, \
            tc.tile_pool(name="xin", bufs=3) as xin, \
            tc.tile_pool(name="zt", bufs=3) as ztp, \
            tc.tile_pool(name="vp", bufs=2 * NKT) as vpp, \
            tc.tile_pool(name="pt", bufs=9) as ptp, \
            tc.tile_pool(name="outs", bufs=4) as outp, \
            tc.tile_pool(name="st", bufs=2, space="PSUM") as stp, \
            tc.tile_pool(name="av", bufs=3, space="PSUM") as avp, \
            tc.tile_pool(name="vq", bufs=1, space="PSUM") as vqp:

        c_u = consts.tile([F, HPC], f32, tag="u")
        nc.gpsimd.dma_start(out=c_u, in_=ud[:, :])
        # allocated here, DMA'd after head 0's x quarter 0 (below) so
        # the first strip's data goes out on the gpsimd queue first
        c_mask = consts.tile([F, F], bf16, tag="msk")
        c_vb = consts.tile([128, GSTRIDE + VW], bf16, tag="vb")
        # touch Exp once so ACT's table set loads during the input DMAs
        # instead of on the first real softmax strip
        warm = consts.tile([1, 8], f32, tag="warm")
        nc.vector.memset(warm[:, 0:8], 0.0)
        nc.scalar.activation(out=warm[:, 0:8], in_=warm[:, 0:8],
                             func=Exp)

        # deferred AV-batch emission: by the time an AV batch is
        # emitted, the exp it reads finished ~2 iterations ago, so the
        # PE never stalls waiting on ACT
        SKEW = 5
        pending = []

        def flush_pending(keep=0):
            while len(pending) > keep:
                pending.pop(0)()

        def make_prelude(hd):
            """Emission closures for head hd's input DMAs, Z and V'
            projections. Popped one-per-ki during head hd-1's k-loop so
            this work hides under the previous head's softmax."""
            st8 = {"vav": [], "vbeta": []}

            def dmas(hd=hd):
                mt = xin.tile([F, F], bf16, tag="mt", name=f"mt_{hd}")
                nc.sync.dma_start(out=mt, in_=mtd[hd])
                xbh = xin.tile([F, S], bf16, tag="xbh", name=f"xbh_{hd}")
                wv = xin.tile([F, VW], bf16, tag="wv", name=f"wv_{hd}")
                if hd == 0:
                    # startup is DMA-latency bound: spread the first
                    # quarters across the idle gpsimd + sync queues so
                    # the z0 -> strip0 -> exp chain starts ~4us earlier
                    nc.gpsimd.dma_start(out=xbh[:, 0:512],
                                        in_=xtb[hd][:, 0:512])
                    nc.sync.dma_start(out=xbh[:, 512:1024],
                                      in_=xtb[hd][:, 512:1024])
                    nc.sync.dma_start(out=wv, in_=wvx[hd])
                    nc.gpsimd.dma_start(out=c_mask, in_=msk[:, :])
                    nc.gpsimd.dma_start(out=c_vb, in_=vb1[:, :])
                    nc.sync.dma_start(out=xbh[:, 1024:1536],
                                      in_=xtb[hd][:, 1024:1536])
                    nc.sync.dma_start(out=xbh[:, 1536:2048],
                                      in_=xtb[hd][:, 1536:2048])
                else:
                    nc.sync.dma_start(out=xbh[:, 0:HALF],
                                      in_=xtb[hd][:, 0:HALF])
                    nc.sync.dma_start(out=wv, in_=wvx[hd])
                    nc.sync.dma_start(out=xbh[:, HALF:S],
                                      in_=xtb[hd][:, HALF:S])
                st8["xbh"], st8["mt"], st8["wv"] = xbh, mt, wv
                st8["zt"] = ztp.tile([F, S], bf16, tag="zt", name=f"zt_{hd}")

            def z_chunk(c, hd=hd, pool=None, tag="vq", act=False):
                ps = (pool or vqp).tile([128, 512], f32, tag=tag,
                                        name=f"z_{hd}_{c}")
                nc.tensor.matmul(
                    ps[:, 0:512], st8["mt"][:, :],
                    st8["xbh"][:, 512 * c:512 * (c + 1)],
                    start=True, stop=True)
                dst = st8["zt"][:, 512 * c:512 * (c + 1)]
                if act:  # startup only: ACT is idle then
                    nc.scalar.activation(out=dst, in_=ps[:, 0:512],
                                         func=Ident, bias=c_u[:, hd:hd + 1])
                else:
                    nc.vector.tensor_scalar_add(dst, ps[:, 0:512],
                                                c_u[:, hd:hd + 1])

            def vpd_tile(j, hd=hd):
                # two s-tiles of V' share one PSUM bank (cols 0 and
                # GSTRIDE) and one SBUF tile + one evacuation copy. The
                # second prefill's start=True clears the whole bank's
                # has_written, but pair A is fully written by then
                # (data persists).
                ps = vqp.tile([128, 512], f32, tag="vq",
                              name=f"vps_{hd}_{j}")
                for half_j in range(2):
                    si = 2 * j + half_j
                    g = GSTRIDE * half_j
                    nc.tensor.matmul(
                        ps[:, g:g + VW],
                        st8["xbh"][:, 128 * si:128 * (si + 1)],
                        st8["wv"][:, :],
                        start=True, stop=True, skip_group_check=True)
                vt = vpp.tile([128, GSTRIDE + VW], bf16, tag="vp",
                              name=f"vp_{hd}_{j}")
                # evacuation copy with the denominator-ones column
                # folded in via the broadcast bias tile
                nc.vector.scalar_tensor_tensor(
                    out=vt[:, :], in0=ps[:, 0:GSTRIDE + VW], scalar=1.0,
                    in1=c_vb[:, :], op0=mybir.AluOpType.mult,
                    op1=mybir.AluOpType.add)
                for half_j in range(2):
                    g = GSTRIDE * half_j
                    st8["vav"].append(vt[:, g:g + F + 1])
                    st8["vbeta"].append(vt[:, g + F + 1:g + F + 2])

            # ordered so V' pairs arrive ahead of the exps that read
            # their beta column, and Z chunks ahead of the strips that
            # read them; 13 closures <= 24 k-iterations
            closures = [dmas]
            if hd == 0:
                # startup: half0 strips 0-3 need only z chunk 0 and x
                # quarters 0-1; order pops so nothing head-of-line
                # blocks the PE queue on a not-yet-arrived x quarter
                closures.append(lambda: z_chunk(0, pool=stp, tag="st",
                                                act=True))
                closures.append(lambda: vpd_tile(0))
                order = [lambda: vpd_tile(1), lambda: z_chunk(1),
                         lambda: vpd_tile(2), lambda: vpd_tile(3),
                         lambda: z_chunk(2), lambda: z_chunk(3),
                         lambda: vpd_tile(4), lambda: vpd_tile(5),
                         lambda: vpd_tile(6), lambda: vpd_tile(7)]
            else:
                closures.append(lambda: z_chunk(0))
                closures.append(lambda: z_chunk(1))
                closures.append(lambda: vpd_tile(0))
                order = [lambda: z_chunk(2), lambda: vpd_tile(1),
                         lambda: z_chunk(3), lambda: vpd_tile(2),
                         lambda: vpd_tile(3), lambda: vpd_tile(4),
                         lambda: vpd_tile(5), lambda: vpd_tile(6),
                         lambda: vpd_tile(7)]
            closures.extend(order)
            return st8, closures

        head_state = {}
        head_state[0], prelude = make_prelude(0)
        for _ in range(3):  # dmas + z0 + vpd0; rest pops in the k-loop
            prelude.pop(0)()
        total_iters = HPC * 24  # for the end-of-kernel pending drain
        it = 0

        for hd in range(HPC):
            if hd > 0:
                while prelude:  # leftovers from the previous k-loop
                    prelude.pop(0)()
            if hd + 1 < HPC:
                head_state[hd + 1], nxt = make_prelude(hd + 1)
                prelude.extend(nxt)
            zt_t = head_state[hd]["zt"]
            xbh_t = head_state[hd]["xbh"]
            vav = head_state[hd]["vav"]
            vbeta = head_state[hd]["vbeta"]

            # --- attention, q in two 1024-wide halves ---
            for half in range(2):
                q0 = half * HALF
                nk = (half + 1) * (HALF // 128)  # k-tiles for this half
                hstate = {}

                for ki in range(nk):
                    ks = 128 * ki
                    ls = max(0, ks - q0)  # local start col within strip
                    strip = stp.tile([128, 1024], f32, tag="st")
                    bounds = [ls, 512, 1024] if ls < 512 else [ls, 1024]
                    pieces = list(zip(bounds[:-1], bounds[1:]))
                    # both ST pieces first so exp can start as early as
                    # possible; the deferred AV batch then streams on
                    # the PE while ACT runs this strip's exp
                    for c0, c1 in pieces:
                        nc.tensor.matmul(
                            strip[:, c0:c1], zt_t[:, ks:ks + 128],
                            xbh_t[:, q0 + c0:q0 + c1],
                            start=True, stop=True)
                    ptile = ptp.tile([128, 1024], bf16, tag="pt")
                    nc.scalar.activation(
                        out=ptile[:, ls:1024], in_=strip[:, ls:1024],
                        func=Exp, scale=SCALE, bias=vbeta[ki])
                    if ks >= q0:  # zero below-diagonal of the diag block
                        nc.vector.tensor_mul(
                            ptile[:, ls:ls + 128], ptile[:, ls:ls + 128],
                            c_mask[:, :])
                    if prelude:  # hide next head's Z/V' here
                        prelude.pop(0)()
                    it += 1
                    keep = min(SKEW - 1, total_iters - it)
                    flush_pending(keep=keep)

                    def av_batch(hd=hd, half=half, ki=ki, ptile=ptile,
                                 hstate=hstate, vav=vav):
                        if ki == 0:
                            # start=True clears has_written for the
                            # WHOLE bank (per partition), so only the
                            # FIRST matmul into each bank (qt%3==0 at
                            # ki=0) may carry it; the other packed
                            # groups' first writes find their bits
                            # clear and overwrite.
                            hstate["avts"] = [
                                avp.tile([128, 512], f32, tag="av",
                                         name=f"avacc_{hd}_{half}_{i}")
                                for i in range(3)]
                        avts = hstate["avts"]
                        for qt in range(max(0, ki - 8 * half), 8):
                            qg = 8 * half + qt
                            g = GSTRIDE * (qt % 3)
                            acc = avts[qt // 3][:, g:g + F + 1]
                            nc.tensor.matmul(
                                acc, ptile[:, 128 * qt:128 * qt + 128],
                                vav[ki][:, :],
                                start=(ki == 0 and qt % 3 == 0),
                                stop=(ki == qg),
                                skip_group_check=True)
                        # once a whole accumulator bank is finished,
                        # stage it to SBUF with ONE copy (DMA cannot
                        # read PSUM) and DMA the (unnormalized) groups
                        # out in ONE 3D-AP transfer; host divides by
                        # the denominator column
                        last_head = hd == HPC - 1 and half == 1
                        for bank in range(3):
                            last_qt = min(3 * bank + 2, 7)
                            if ki != 8 * half + last_qt:
                                continue
                            ng = last_qt - 3 * bank + 1
                            w = GSTRIDE * (ng - 1) + F + 1
                            stage = outp.tile([128, 3 * GSTRIDE], f32,
                                              tag="ot")
                            nc.vector.tensor_copy(
                                out=stage[:, 0:w],
                                in_=avts[bank][:, 0:w])
                            qg0 = 8 * half + 3 * bank
                            if last_head:
                                # end of kernel: spread single-group
                                # DMAs across now-idle queues so the
                                # final drain is parallel and short
                                engs = [nc.sync, nc.gpsimd, nc.scalar]
                                for j in range(ng):
                                    g = GSTRIDE * j
                                    engs[j % 3].dma_start(
                                        out=out[hd, qg0 + j],
                                        in_=stage[:, g:g + F + 1])
                            else:
                                src = stage[:, 0:GSTRIDE * ng].rearrange(
                                    "p (g c) -> p g c",
                                    c=GSTRIDE)[:, :, 0:F + 1]
                                dst = out[hd, qg0:qg0 + ng].transpose(
                                    [1, 0, 2])
                                eng = nc.gpsimd if (bank & 1) else nc.sync
                                eng.dma_start(out=dst, in_=src)

                    pending.append(av_batch)
        flush_pending()

    nc.compile()
    return nc


def _prep_inputs(x, Wq, Wk, Wv, bq, bk, bv):
    """Shard + pre-transpose + fold weights on host. 8 core in_maps."""
    bf16 = ml_dtypes.bfloat16
    xf = np.ascontiguousarray(
        x.reshape(B * H, S, F).transpose(0, 2, 1))          # [32, F, S]
    xfb = xf.astype(bf16)
    # mt = M^T = (Wq^T Wk)^T = Wk^T Wq, per head  [f, f']
    mt = np.einsum("hef,heg->hfg", Wk, Wq).astype(bf16)     # [H, f, g=f']
    u = np.einsum("hef,he->hf", Wq, bk).astype(np.float32)  # [H, f']
    w = np.einsum("hef,he->hf", Wk, bq).astype(np.float32)  # [H, f]
    # wvx = [Wv^T | 0 | SCALE*w]  [f, VW]
    wvxh = np.zeros((H, F, VW), np.float32)
    wvxh[:, :, :F] = Wv.transpose(0, 2, 1)
    wvxh[:, :, F + 1] = SCALE * w
    wvxh = wvxh.astype(bf16)
    vb = np.zeros((128, GSTRIDE + VW), np.float32)
    vb[:, F] = 1.0
    vb[:, GSTRIDE + F] = 1.0
    mask = np.triu(np.ones((F, F), np.float32)).astype(bf16)  # keep r <= c

    in_maps = []
    for c in range(NCORES):
        pairs = list(range(HPC * c, HPC * (c + 1)))
        heads = [p % H for p in pairs]
        m = {
            "xtbh": np.ascontiguousarray(xfb[pairs]),
            "mtd": np.ascontiguousarray(mt[heads]),
            "wvx": np.ascontiguousarray(wvxh[heads]),
            "ud": np.ascontiguousarray(u[heads].T).astype(np.float32),
            "vb1": vb.astype(bf16),
            "msk": mask,
        }
        in_maps.append(m)
    return in_maps


def kernel(x, Wq, Wk, Wv, bq, bk, bv, trace=False):
    x, Wq, Wk, Wv = (np.asarray(a, np.float32) for a in (x, Wq, Wk, Wv))
    bq, bk, bv = (np.asarray(a, np.float32) for a in (bq, bk, bv))

    if "nc" not in _cache:
        _cache["nc"] = _build()
    nc = _cache["nc"]

    in_maps = _prep_inputs(x, Wq, Wk, Wv, bq, bk, bv)
    res = bass_utils.run_bass_kernel_spmd(
        nc, in_maps, core_ids=list(range(NCORES)), trace=trace)

    out = np.empty((B * H, S, F), np.float32)
    for c in range(NCORES):
        pairs = range(HPC * c, HPC * (c + 1))
        r = res.results[c]["out"]  # [HPC, NKT, 128, 129] unnormalized
        for i, p in enumerate(pairs):
            acc = r[i].reshape(S, F + 1)
            out[p] = acc[:, :F] / acc[:, F:F + 1] + bv[p % H]
    full = out.reshape(B, H, S, F)
    if trace:
        return full, res
    return full
